# revision 53
# baseline (speedup 1.0000x reference)
"""Tensor-parallel attention kernel for Trainium2 (8 NeuronCores).

Problem: B=1, L=2048, D=4096, H=32 q-heads, KV=8 kv-heads, HD=128,
partial rotary ROT=64, causal additive mask, o-projection.

Sharding: TP-8 over heads. Core c owns q-heads 4c..4c+3 and kv-head c
(column shard of w_qkv), plus the matching row shard of w_o. Each core
computes a full [L, D] partial of the output; the host sums the 8
partials (the cross-core reduction of the row-sharded o-projection).

All on-chip tensors are fp16 (PE runs fp16 at the same 1 row/cycle as
fp32r but DMA and SBUF cost halve; fp8 fails the accuracy budget).
Everything runs in "transposed" orientation so every matmul contracts
over the partition dim:
  qkvT[col, L] = w_qkv.T @ x.T          (w stationary, xT streamed)
  rope:  qT' = qT * cosE + (P @ qT) * sinE   (P = rotate-half matrix on PE)
  ST[k, q]   = kT_tile.T @ qT            (roped q kept in SBUF)
  PT         = exp(ST + maskT - 4)       (global bias keeps PT in fp16 range)
  den[*, q]  = ones.T @ PT               (ones-matmul, accumulated over k)
  oT[d, q]   = V_tile.T @ PT             (V from a one-time PE transpose of vT)
  out[l, e]  = (oT/den).T @ w_o_shard    (partial; summed across cores on host)

PE-stall avoidance:
  - w_qkv is loaded in column-group chunks over 3 DMA queues so the
    first accumulation group only waits on 1/6 of the weights.
  - phase-1 rope matmuls / V transposes are deferred one group so the
    PE never waits on the Act-engine PSUM->SBUF copy.
  - the attention den/O matmuls run one k-pair behind the S matmuls and
    the pipeline carries across head and q-block boundaries.
  - the o-projection for a 512-row q-block is issued right after its 4
    heads finish attention (w_o fully SBUF-resident), so output DMA is
    spread over the whole back half of the kernel.
"""

import sys

for _p in ("/opt/trn_rl_repo", "/root/.axon_site/_ro/trn_rl_repo"):
    if _p not in sys.path:
        sys.path.append(_p)

import numpy as np

B, L, D = 1, 2048, 4096
H, KV, HD = 32, 8, 128
ROT = 64
SCALE = HD ** -0.5
NEG = -1e9
MASK_NEG = -30000.0        # fp16-safe stand-in for -1e9
EXP_BIAS = -4.0            # exp(s-4): keeps unnormalized P well inside fp16
NCORES = 8
HPC = H // NCORES          # q-heads per core (4)
NCT = HPC + 2              # 128-wide column groups per core (q*4, k, v)
CPC = NCT * 128            # w_qkv columns per core (768)
NDT = D // 128             # contraction tiles over D (32)
NKT = L // 128             # k tiles (16)
NJQ = L // 512             # 512-wide q blocks (4)
XBLK = 512                 # L-block width in the qkv phase

_cache = {}


def _build(causal: bool):
    import concourse.mybir as mybir
    import concourse.tile as tile
    from concourse import bacc

    F32 = mybir.dt.float32
    F16 = mybir.dt.float16
    EXP = mybir.ActivationFunctionType.Exp
    AXX = mybir.AxisListType.X
    ADD = mybir.AluOpType.add

    nc = bacc.Bacc("TRN2", target_bir_lowering=False, debug=False)

    xt = nc.dram_tensor("xt", [D, L], F16, kind="ExternalInput").ap()
    # w_qkv pre-permuted on host: [p, ct, dt, c] so a column-group chunk
    # is one contiguous 8KB-per-partition DMA
    wqkv = nc.dram_tensor("wqkv", [128, NCT, NDT, 128], F16,
                          kind="ExternalInput").ap()
    wo = nc.dram_tensor("wo", [HPC * HD, D], F16, kind="ExternalInput").ap()
    cos_e = nc.dram_tensor("cos_e", [2, 128, L], F16, kind="ExternalInput").ap()
    sin_e = nc.dram_tensor("sin_e", [2, 128, L], F16, kind="ExternalInput").ap()
    consts = nc.dram_tensor("consts", [128, 385], F16, kind="ExternalInput").ap()
    if causal:
        # block-diagonal strip of maskT: [jq, ktile-in-block, 128, 512]
        mask_d = nc.dram_tensor("mask_d", [NJQ, 4, 128, 512], F16,
                                kind="ExternalInput").ap()
    else:
        mask_t = nc.dram_tensor("mask_t", [L, L], F16, kind="ExternalInput").ap()
    out_p = nc.dram_tensor("out_p", [L, D], F16, kind="ExternalOutput").ap()

    xt_r = xt.rearrange("(dt p) l -> p dt l", p=128)
    wo_r = wo.rearrange("(h p) e -> p h e", p=128)

    with tile.TileContext(nc) as tc:
        with tc.tile_pool(name="persist", bufs=1) as persist:
            kt_sb = persist.tile([128, L], F16, tag="kt")
            v_sb = persist.tile([128, NKT, 128], F16, tag="v")
            qt_sb = persist.tile([128, HPC, L], F16, tag="qt")
            cst = persist.tile([128, 385], F16, tag="cst")
            nc.sync.dma_start(out=cst, in_=consts)
            ident = cst[:, 0:128]
            ones = cst[:, 128:256]
            pmat_t = cst[:, 256:384]
            bias_col = cst[:, 384:385]

            # preload pools: w_o (+ causal mask strip), resident through the
            # whole kernel; DMAs triggered early on the sync ring, behind
            # only the first w_qkv chunks
            wop_cm = tc.tile_pool(name="wob", bufs=1)
            mbp_cm = tc.tile_pool(name="mb", bufs=1 if causal else 2)
            wop = wop_cm.__enter__()
            mbp = mbp_cm.__enter__()
            wo_sb = wop.tile([128, HPC, D], F16)
            mall = None
            if causal:
                mall = mbp.tile([128, 4 * NJQ, 512], F16, tag="mall")

            # ---------------- Phase 1: qkv projection + rope ----------------
            with tc.tile_pool(name="wq", bufs=1) as wqp, \
                 tc.tile_pool(name="xb", bufs=2) as xbp, \
                 tc.tile_pool(name="tabs", bufs=1) as tabs, \
                 tc.tile_pool(name="stage", bufs=3) as stage, \
                 tc.tile_pool(name="vtmp", bufs=1) as vtmp, \
                 tc.tile_pool(name="ps1", bufs=4, space="PSUM") as ps1, \
                 tc.tile_pool(name="psr", bufs=2, space="PSUM") as psr:
                wq_sb = wqp.tile([128, NCT, NDT, 128], F16)
                cos_sb = tabs.tile([128, 2, L], F16, tag="cos")
                sin_sb = tabs.tile([128, 2, L], F16, tag="sin")

                pend = []  # deferred PE work (rope matmul / V transpose)

                def flush():
                    for f in pend:
                        f()
                    pend.clear()

                for lb in range(L // XBLK):
                    ls = slice(lb * XBLK, (lb + 1) * XBLK)
                    xblk = xbp.tile([128, NDT, XBLK], F16, tag="xblk")
                    if lb == 0:
                        # balance the startup traffic over all 3 rings in PE
                        # consumption order: ct0 weights + all x slabs first
                        def slab(sl, eng):
                            ss = slice(sl * NDT // 8, (sl + 1) * NDT // 8)
                            eng.dma_start(out=xblk[:, ss, :], in_=xt_r[:, ss, ls])

                        def wq_half(ct, hf, eng):
                            hs = slice(hf * NDT // 2, (hf + 1) * NDT // 2)
                            eng.dma_start(out=wq_sb[:, ct, hs], in_=wqkv[:, ct, hs])

                        wq_half(0, 0, nc.sync)
                        wq_half(0, 1, nc.scalar)
                        for sl in range(4):
                            slab(sl, nc.gpsimd)
                        slab(4, nc.sync)
                        slab(5, nc.scalar)
                        slab(6, nc.sync)
                        slab(7, nc.scalar)
                        for ct in range(1, NCT):
                            wq_half(ct, 0, nc.sync)
                            wq_half(ct, 1, nc.scalar)
                        # rope tables, then mask strip + w_o, behind the
                        # weights on the sync ring
                        nc.sync.dma_start(
                            out=cos_sb, in_=cos_e.rearrange("t p l -> p t l"))
                        nc.scalar.dma_start(
                            out=sin_sb, in_=sin_e.rearrange("t p l -> p t l"))
                        if causal:
                            nc.sync.dma_start(
                                out=mall,
                                in_=mask_d.rearrange("jq kt p q -> p (jq kt) q"))
                        for eh in range(4):
                            es = slice(eh * (D // 4), (eh + 1) * (D // 4))
                            nc.sync.dma_start(out=wo_sb[:, :, es], in_=wo_r[:, :, es])
                    elif lb == 1:
                        # split across the two rings that free up first
                        h1 = NDT // 2
                        nc.gpsimd.dma_start(out=xblk[:, :h1, :], in_=xt_r[:, :h1, ls])
                        nc.scalar.dma_start(out=xblk[:, h1:, :], in_=xt_r[:, h1:, ls])
                    else:
                        eng = nc.gpsimd if lb == 2 else nc.scalar
                        eng.dma_start(out=xblk, in_=xt_r[:, :, ls])
                    for ct in range(NCT):
                        acc = ps1.tile([128, XBLK], F32, tag="acc")
                        for dti in range(NDT):
                            nc.tensor.matmul(
                                out=acc,
                                lhsT=wq_sb[:, ct, dti, :],
                                rhs=xblk[:, dti, :],
                                start=(dti == 0), stop=(dti == NDT - 1))
                        flush()
                        if ct == 5:
                            # v: copy to vT staging; transpose deferred
                            vt_sb = vtmp.tile([128, XBLK], F16, tag="vt")
                            nc.scalar.copy(out=vt_sb, in_=acc)

                            def vtrans(lb=lb, vt_sb=vt_sb):
                                for kk in range(XBLK // 128):
                                    i = (XBLK // 128) * lb + kk
                                    tp = psr.tile([128, 128], F16, tag="vtp")
                                    nc.tensor.transpose(
                                        tp, vt_sb[:, kk * 128:(kk + 1) * 128], ident)
                                    nc.vector.tensor_copy(v_sb[:, i, :], tp)

                            pend.append(vtrans)
                            continue
                        # rope for q (ct 0..3, scaled tables) and k (ct 4)
                        ti = 0 if ct < 4 else 1
                        s_sb = stage.tile([128, XBLK], F16, tag="s_sb")
                        nc.scalar.copy(out=s_sb, in_=acc)

                        def rope(ct=ct, ti=ti, ls=ls, s_sb=s_sb):
                            rot = psr.tile([128, XBLK], F32, tag="rot")
                            nc.tensor.matmul(out=rot, lhsT=pmat_t, rhs=s_sb,
                                             start=True, stop=True)
                            dtile = kt_sb[:, ls] if ct == 4 else qt_sb[:, ct, ls]
                            nc.vector.tensor_mul(dtile, s_sb, cos_sb[:, ti, ls])
                            m2 = stage.tile([128, XBLK], F16, tag="m2")
                            nc.vector.tensor_mul(m2, rot, sin_sb[:, ti, ls])
                            nc.vector.tensor_add(dtile, dtile, m2)

                        pend.append(rope)
                flush()

            # ---------- Phases 2+3: attention + o-projection, interleaved ----
            with tc.tile_pool(name="otn", bufs=2) as otnp, \
                 tc.tile_pool(name="pt", bufs=4) as ptp, \
                 tc.tile_pool(name="rdp", bufs=2) as rdp, \
                 tc.tile_pool(name="psm", bufs=2) as psmp, \
                 tc.tile_pool(name="ost", bufs=8) as ostp, \
                 tc.tile_pool(name="ps_st", bufs=2, space="PSUM") as ps_st, \
                 tc.tile_pool(name="ps_acc", bufs=1, space="PSUM") as ps_acc, \
                 tc.tile_pool(name="ps3", bufs=2, space="PSUM") as ps3:
                pending = []  # deferred den/O matmuls, up to 3 k-pairs deep

                def den_o_flush(all_=False):
                    while pending and (all_ or len(pending) > 3):
                        pending.pop(0)()

                out_q = [0]

                def out_queue():
                    q = (nc.sync, nc.gpsimd, nc.scalar)[out_q[0] % 3]
                    out_q[0] += 1
                    return q

                def emit_oproj(jq, otn_sb, final=False):
                    # o-projection for q block jq (its attention is done)
                    for lt in range(4 * jq, 4 * jq + 4):
                        lo = (lt - 4 * jq) * 128
                        for et in range(D // 512):
                            es = slice(et * 512, (et + 1) * 512)
                            acc = ps3.tile([128, 512], F32, tag="acc3")
                            for h in range(HPC):
                                nc.tensor.matmul(
                                    out=acc,
                                    lhsT=otn_sb[:, h, lo:lo + 128],
                                    rhs=wo_sb[:, h, es],
                                    start=(h == 0), stop=(h == HPC - 1))
                            # copies stay off the Act engine while exps still
                            # pace the attention pipeline; the final block has
                            # no exps left, so split copies DVE/Act to halve
                            # the end-of-kernel drain
                            ost = ostp.tile([128, 512], F16, tag="ost")
                            if final and et % 2 == 1:
                                nc.scalar.copy(out=ost, in_=acc)
                            else:
                                nc.vector.tensor_copy(ost, acc)
                            out_queue().dma_start(
                                out=out_p[lt * 128:(lt + 1) * 128, es], in_=ost)

                oproj_q = []  # q blocks whose o-projection is still pending

                for jq in range(NJQ):
                    qs = slice(jq * 512, (jq + 1) * 512)
                    nkt = 4 * (jq + 1) if causal else NKT
                    npair = nkt // 2
                    diag0 = 4 * jq
                    if causal:
                        mblk = mall[:, 4 * jq:4 * jq + 4, :]
                    else:
                        mblk = mbp.tile([128, NKT, 512], F16, tag="mblk")
                        nc.sync.dma_start(
                            out=mblk,
                            in_=mask_t[:, qs].rearrange("(kt p) q -> p kt q", p=128))
                    otn_sb = otnp.tile([128, HPC, 512], F16, tag="otn")
                    for h in range(HPC):
                        qblk = qt_sb[:, h, qs]
                        den = ps_acc.tile([128, 512], F32, tag="den")
                        ot = ps_acc.tile([128, 512], F32, tag="ot")
                        for j in range(npair):
                            st = ps_st.tile([128, 2, 512], F32, tag="st")
                            pt = ptp.tile([128, 2, 512], F16, tag="pt")
                            for jj in range(2):
                                i = 2 * j + jj
                                nc.tensor.matmul(
                                    out=st[:, jj, :],
                                    lhsT=kt_sb[:, i * 128:(i + 1) * 128],
                                    rhs=qblk, start=True, stop=True)
                            for jj in range(2):
                                i = 2 * j + jj
                                if causal:
                                    if i >= diag0:
                                        nc.vector.tensor_add(
                                            st[:, jj, :], st[:, jj, :],
                                            mblk[:, i - diag0, :])
                                else:
                                    nc.vector.tensor_add(
                                        st[:, jj, :], st[:, jj, :], mblk[:, i, :])
                            nc.scalar.activation(pt, st, EXP, bias=bias_col)
                            if oproj_q:
                                # previous q block's o-projection rides in the
                                # attention pipeline gap; its otn must be fully
                                # written, so drain the den/O queue first
                                den_o_flush(all_=True)
                                emit_oproj(*oproj_q.pop(0))

                            def den_o(pt=pt, j=j, den=den, ot=ot, nkt=nkt,
                                      last=(j == npair - 1), h=h, otn_sb=otn_sb):
                                for jj in range(2):
                                    i = 2 * j + jj
                                    nc.tensor.matmul(
                                        out=den, lhsT=ones, rhs=pt[:, jj, :],
                                        start=(i == 0), stop=(i == nkt - 1))
                                    nc.tensor.matmul(
                                        out=ot, lhsT=v_sb[:, i, :], rhs=pt[:, jj, :],
                                        start=(i == 0), stop=(i == nkt - 1))
                                if last:
                                    rd = rdp.tile([128, 512], F32, tag="rd")
                                    nc.vector.reciprocal_approx_fast(out=rd, in_=den)
                                    nc.vector.tensor_mul(otn_sb[:, h, :], ot, rd)

                            pending.append(den_o)
                            den_o_flush()

                    oproj_q.append((jq, otn_sb))

                den_o_flush(all_=True)
                emit_oproj(*oproj_q.pop(0), final=True)

            mbp_cm.__exit__(None, None, None)
            wop_cm.__exit__(None, None, None)

    nc.compile()
    return nc


def _host_inputs(x, attention_mask, cos, sin, w_qkv, w_o, causal):
    """Build the 8 per-core input maps (fp16, C-contiguous)."""
    F16 = np.float16
    xt = np.ascontiguousarray(x[0].T).astype(F16)         # [D, L]
    q_pos = H * HD
    kv_pos = q_pos + KV * HD

    # extended rope tables [2, 128, L]: slot 0 = q (scale folded), slot 1 = k
    # row d<64: cos[l, d]; row d>=64: 1.0 (cos) / 0.0 (sin)
    cos_t = cos.T.astype(np.float32)                      # [ROT, L]
    sin_t = sin.T.astype(np.float32)
    cos_e = np.empty((2, 128, L), np.float32)
    sin_e = np.zeros((2, 128, L), np.float32)
    cos_e[0, :ROT] = cos_t * SCALE
    cos_e[0, ROT:] = SCALE
    cos_e[1, :ROT] = cos_t
    cos_e[1, ROT:] = 1.0
    sin_e[0, :ROT] = sin_t * SCALE
    sin_e[1, :ROT] = sin_t
    cos_e = cos_e.astype(F16)
    sin_e = sin_e.astype(F16)

    # consts [128, 385] = [identity | ones | pmat_t | exp bias]
    # pmat_t[d, d'] = Pmat[d', d]; rot[d'] = -x[d'+32] (d'<32), x[d'-32] (32<=d'<64)
    pmat = np.zeros((128, 128), np.float32)
    for dp in range(32):
        pmat[dp, dp + 32] = -1.0
    for dp in range(32, 64):
        pmat[dp, dp - 32] = 1.0
    consts = np.concatenate(
        [np.eye(128, dtype=np.float32), np.ones((128, 128), np.float32), pmat.T,
         np.full((128, 1), EXP_BIAS, np.float32)],
        axis=1).astype(F16)

    # fp16-safe mask: clamp -1e9 to -30000 (exp still exactly 0 after bias)
    mask2d = np.maximum(attention_mask[0, 0], MASK_NEG)   # [L(q), L(k)]
    if causal:
        mask_t_full = None
        # diagonal 512x512 blocks of maskT, split into 128-row k strips
        mask_d = np.empty((NJQ, 4, 128, 512), F16)
        mt = mask2d.T                                     # [k, q]
        for jq in range(NJQ):
            blk = mt[jq * 512:(jq + 1) * 512, jq * 512:(jq + 1) * 512]
            mask_d[jq] = blk.reshape(4, 128, 512)
        mask_d = np.ascontiguousarray(mask_d)
    else:
        mask_t_full = np.ascontiguousarray(mask2d.T).astype(F16)  # [k, q]
        mask_d = None

    in_maps = []
    for c in range(NCORES):
        cols = []
        for j in range(HPC):
            h = c * HPC + j
            cols.append(w_qkv[:, h * HD:(h + 1) * HD])
        cols.append(w_qkv[:, q_pos + c * HD:q_pos + (c + 1) * HD])
        cols.append(w_qkv[:, kv_pos + c * HD:kv_pos + (c + 1) * HD])
        wqkv_c = np.concatenate(cols, axis=1)             # [D, 768]
        # pre-permute to [p, ct, dt, c]: contiguous per-partition chunks
        wqkv_c = np.ascontiguousarray(
            wqkv_c.reshape(NDT, 128, NCT, 128).transpose(1, 2, 0, 3)).astype(F16)
        wo_c = np.ascontiguousarray(
            w_o[c * HPC * HD:(c + 1) * HPC * HD, :]).astype(F16)     # [512, D]
        m = {"xt": xt, "wqkv": wqkv_c, "wo": wo_c,
             "cos_e": cos_e, "sin_e": sin_e, "consts": consts}
        if causal:
            m["mask_d"] = mask_d
        else:
            m["mask_t"] = mask_t_full
        in_maps.append(m)
    return in_maps


def _is_causal(mask2d):
    expected = np.where(
        np.tril(np.ones((L, L), dtype=bool)), np.float32(0.0), np.float32(NEG))
    return mask2d.shape == (L, L) and np.array_equal(mask2d, expected)


def kernel(x, attention_mask, cos, sin, w_qkv, w_o, _trace=False):
    from concourse.bass_utils import run_bass_kernel_spmd

    x = np.asarray(x, dtype=np.float32)
    attention_mask = np.asarray(attention_mask, dtype=np.float32)
    cos = np.asarray(cos, dtype=np.float32)
    sin = np.asarray(sin, dtype=np.float32)
    w_qkv = np.asarray(w_qkv, dtype=np.float32)
    w_o = np.asarray(w_o, dtype=np.float32)

    causal = _is_causal(attention_mask[0, 0])
    if causal not in _cache:
        _cache[causal] = _build(causal)
    nc = _cache[causal]

    in_maps = _host_inputs(x, attention_mask, cos, sin, w_qkv, w_o, causal)
    try:
        res = run_bass_kernel_spmd(nc, in_maps, list(range(NCORES)), trace=_trace)
    except Exception:
        # transient device errors (e.g. NRT_EXEC_UNIT_UNRECOVERABLE) usually
        # clear on retry
        res = run_bass_kernel_spmd(nc, in_maps, list(range(NCORES)), trace=_trace)
    out = np.zeros((L, D), np.float64)
    for c in range(NCORES):
        out += res.results[c]["out_p"].astype(np.float64)
    if _trace:
        kernel._last_exec_time_ns = res.exec_time_ns
    return out.astype(np.float32).reshape(B, L, D)


# revision 56
# speedup vs baseline: 1.0102x; 1.0102x over previous
"""Tensor-parallel attention kernel for Trainium2 (8 NeuronCores).

Problem: B=1, L=2048, D=4096, H=32 q-heads, KV=8 kv-heads, HD=128,
partial rotary ROT=64, causal additive mask, o-projection.

Sharding: TP-8 over heads. Core c owns q-heads 4c..4c+3 and kv-head c
(column shard of w_qkv), plus the matching row shard of w_o. Each core
computes a full [L, D] partial of the output; the host sums the 8
partials (the cross-core reduction of the row-sharded o-projection).

All on-chip tensors are fp16 (PE runs fp16 at the same 1 row/cycle as
fp32r but DMA and SBUF cost halve; fp8 fails the accuracy budget).
Everything runs in "transposed" orientation so every matmul contracts
over the partition dim:
  qkvT[col, L] = w_qkv.T @ x.T          (w stationary, xT streamed)
  rope:  qT' = qT * cosE + (P @ qT) * sinE   (P = rotate-half matrix on PE)
  ST[k, q]   = kT_tile.T @ qT            (roped q kept in SBUF)
  PT         = exp(ST + maskT - 4)       (global bias keeps PT in fp16 range)
  den[*, q]  = ones.T @ PT               (ones-matmul, accumulated over k)
  oT[d, q]   = V_tile.T @ PT             (V from a one-time PE transpose of vT)
  out[l, e]  = (oT/den).T @ w_o_shard    (partial; summed across cores on host)

PE-stall avoidance:
  - w_qkv is loaded in column-group chunks over 3 DMA queues so the
    first accumulation group only waits on 1/6 of the weights.
  - phase-1 rope matmuls / V transposes are deferred one group so the
    PE never waits on the Act-engine PSUM->SBUF copy.
  - the attention den/O matmuls run one k-pair behind the S matmuls and
    the pipeline carries across head and q-block boundaries.
  - the o-projection for a 512-row q-block is issued right after its 4
    heads finish attention (w_o fully SBUF-resident), so output DMA is
    spread over the whole back half of the kernel.
"""

import sys

for _p in ("/opt/trn_rl_repo", "/root/.axon_site/_ro/trn_rl_repo"):
    if _p not in sys.path:
        sys.path.append(_p)

import numpy as np

B, L, D = 1, 2048, 4096
H, KV, HD = 32, 8, 128
ROT = 64
SCALE = HD ** -0.5
NEG = -1e9
MASK_NEG = -30000.0        # fp16-safe stand-in for -1e9
EXP_BIAS = -4.0            # exp(s-4): keeps unnormalized P well inside fp16
NCORES = 8
HPC = H // NCORES          # q-heads per core (4)
NCT = HPC + 2              # 128-wide column groups per core (q*4, k, v)
CPC = NCT * 128            # w_qkv columns per core (768)
NDT = D // 128             # contraction tiles over D (32)
NKT = L // 128             # k tiles (16)
NJQ = L // 512             # 512-wide q blocks (4)
XBLK = 512                 # L-block width in the qkv phase

_cache = {}


def _build(causal: bool):
    import concourse.mybir as mybir
    import concourse.tile as tile
    from concourse import bacc

    F32 = mybir.dt.float32
    F16 = mybir.dt.float16
    EXP = mybir.ActivationFunctionType.Exp
    AXX = mybir.AxisListType.X
    ADD = mybir.AluOpType.add

    nc = bacc.Bacc("TRN2", target_bir_lowering=False, debug=False)

    xt = nc.dram_tensor("xt", [D, L], F16, kind="ExternalInput").ap()
    # w_qkv pre-permuted on host: [p, ct, dt, c] so a column-group chunk
    # is one contiguous 8KB-per-partition DMA
    wqkv = nc.dram_tensor("wqkv", [128, NCT, NDT, 128], F16,
                          kind="ExternalInput").ap()
    wo = nc.dram_tensor("wo", [HPC * HD, D], F16, kind="ExternalInput").ap()
    cos_e = nc.dram_tensor("cos_e", [2, 128, L], F16, kind="ExternalInput").ap()
    sin_e = nc.dram_tensor("sin_e", [2, 128, L], F16, kind="ExternalInput").ap()
    consts = nc.dram_tensor("consts", [128, 385], F16, kind="ExternalInput").ap()
    if causal:
        # block-diagonal strip of maskT: [jq, ktile-in-block, 128, 512]
        mask_d = nc.dram_tensor("mask_d", [NJQ, 4, 128, 512], F16,
                                kind="ExternalInput").ap()
    else:
        mask_t = nc.dram_tensor("mask_t", [L, L], F16, kind="ExternalInput").ap()
    out_p = nc.dram_tensor("out_p", [L, D], F16, kind="ExternalOutput").ap()

    xt_r = xt.rearrange("(dt p) l -> p dt l", p=128)
    wo_r = wo.rearrange("(h p) e -> p h e", p=128)

    with tile.TileContext(nc) as tc:
        with tc.tile_pool(name="persist", bufs=1) as persist:
            kt_sb = persist.tile([128, L], F16, tag="kt")
            v_sb = persist.tile([128, NKT, 128], F16, tag="v")
            qt_sb = persist.tile([128, HPC, L], F16, tag="qt")
            cst = persist.tile([128, 385], F16, tag="cst")
            nc.sync.dma_start(out=cst, in_=consts)
            ident = cst[:, 0:128]
            ones = cst[:, 128:256]
            pmat_t = cst[:, 256:384]
            bias_col = cst[:, 384:385]

            # preload pools: w_o (+ causal mask strip), resident through the
            # whole kernel; DMAs triggered early on the sync ring, behind
            # only the first w_qkv chunks
            wop_cm = tc.tile_pool(name="wob", bufs=1)
            mbp_cm = tc.tile_pool(name="mb", bufs=1 if causal else 2)
            wop = wop_cm.__enter__()
            mbp = mbp_cm.__enter__()
            wo_sb = wop.tile([128, HPC, D], F16)
            mall = None
            if causal:
                mall = mbp.tile([128, 4 * NJQ, 512], F16, tag="mall")

            # ---------------- Phase 1: qkv projection + rope ----------------
            with tc.tile_pool(name="wq", bufs=1) as wqp, \
                 tc.tile_pool(name="xb", bufs=2) as xbp, \
                 tc.tile_pool(name="tabs", bufs=1) as tabs, \
                 tc.tile_pool(name="stage", bufs=3) as stage, \
                 tc.tile_pool(name="vtmp", bufs=1) as vtmp, \
                 tc.tile_pool(name="ps1", bufs=4, space="PSUM") as ps1, \
                 tc.tile_pool(name="psr", bufs=2, space="PSUM") as psr:
                wq_sb = wqp.tile([128, NCT, NDT, 128], F16)
                cos_sb = tabs.tile([128, 2, L], F16, tag="cos")
                sin_sb = tabs.tile([128, 2, L], F16, tag="sin")

                pend = []  # deferred PE work (rope matmul / V transpose)

                def flush():
                    for f in pend:
                        f()
                    pend.clear()

                for lb in range(L // XBLK):
                    ls = slice(lb * XBLK, (lb + 1) * XBLK)
                    xblk = xbp.tile([128, NDT, XBLK], F16, tag="xblk")
                    if lb == 0:
                        # balance the startup traffic over all 3 rings in PE
                        # consumption order: ct0 weights + all x slabs first
                        def slab(sl, eng):
                            ss = slice(sl * NDT // 8, (sl + 1) * NDT // 8)
                            eng.dma_start(out=xblk[:, ss, :], in_=xt_r[:, ss, ls])

                        def wq_half(ct, hf, eng):
                            hs = slice(hf * NDT // 2, (hf + 1) * NDT // 2)
                            eng.dma_start(out=wq_sb[:, ct, hs], in_=wqkv[:, ct, hs])

                        wq_half(0, 0, nc.sync)
                        wq_half(0, 1, nc.scalar)
                        for sl in range(4):
                            slab(sl, nc.gpsimd)
                        slab(4, nc.sync)
                        slab(5, nc.scalar)
                        slab(6, nc.sync)
                        slab(7, nc.scalar)
                        for ct in range(1, NCT):
                            wq_half(ct, 0, nc.sync)
                            wq_half(ct, 1, nc.scalar)
                        # rope tables, then mask strip + w_o, behind the
                        # weights on the sync ring
                        nc.sync.dma_start(
                            out=cos_sb, in_=cos_e.rearrange("t p l -> p t l"))
                        nc.scalar.dma_start(
                            out=sin_sb, in_=sin_e.rearrange("t p l -> p t l"))
                        if causal:
                            nc.sync.dma_start(
                                out=mall,
                                in_=mask_d.rearrange("jq kt p q -> p (jq kt) q"))
                        for eh in range(4):
                            es = slice(eh * (D // 4), (eh + 1) * (D // 4))
                            nc.sync.dma_start(out=wo_sb[:, :, es], in_=wo_r[:, :, es])
                    elif lb == 1:
                        # split across the two rings that free up first
                        h1 = NDT // 2
                        nc.gpsimd.dma_start(out=xblk[:, :h1, :], in_=xt_r[:, :h1, ls])
                        nc.scalar.dma_start(out=xblk[:, h1:, :], in_=xt_r[:, h1:, ls])
                    else:
                        eng = nc.gpsimd if lb == 2 else nc.scalar
                        eng.dma_start(out=xblk, in_=xt_r[:, :, ls])
                    for ct in range(NCT):
                        acc = ps1.tile([128, XBLK], F32, tag="acc")
                        for dti in range(NDT):
                            nc.tensor.matmul(
                                out=acc,
                                lhsT=wq_sb[:, ct, dti, :],
                                rhs=xblk[:, dti, :],
                                start=(dti == 0), stop=(dti == NDT - 1))
                        flush()
                        if ct == 5:
                            # v: copy to vT staging; transpose deferred
                            vt_sb = vtmp.tile([128, XBLK], F16, tag="vt")
                            nc.scalar.copy(out=vt_sb, in_=acc)

                            def vtrans(lb=lb, vt_sb=vt_sb):
                                for kk in range(XBLK // 128):
                                    i = (XBLK // 128) * lb + kk
                                    tp = psr.tile([128, 128], F16, tag="vtp")
                                    nc.tensor.transpose(
                                        tp, vt_sb[:, kk * 128:(kk + 1) * 128], ident)
                                    nc.vector.tensor_copy(v_sb[:, i, :], tp)

                            pend.append(vtrans)
                            continue
                        # rope for q (ct 0..3, scaled tables) and k (ct 4)
                        ti = 0 if ct < 4 else 1
                        s_sb = stage.tile([128, XBLK], F16, tag="s_sb")
                        nc.scalar.copy(out=s_sb, in_=acc)

                        def rope(ct=ct, ti=ti, ls=ls, s_sb=s_sb):
                            rot = psr.tile([128, XBLK], F32, tag="rot")
                            nc.tensor.matmul(out=rot, lhsT=pmat_t, rhs=s_sb,
                                             start=True, stop=True)
                            dtile = kt_sb[:, ls] if ct == 4 else qt_sb[:, ct, ls]
                            nc.vector.tensor_mul(dtile, s_sb, cos_sb[:, ti, ls])
                            m2 = stage.tile([128, XBLK], F16, tag="m2")
                            nc.vector.tensor_mul(m2, rot, sin_sb[:, ti, ls])
                            nc.vector.tensor_add(dtile, dtile, m2)

                        pend.append(rope)
                flush()

            # ---------- Phases 2+3: attention + o-projection, interleaved ----
            with tc.tile_pool(name="otn", bufs=2) as otnp, \
                 tc.tile_pool(name="pt", bufs=4) as ptp, \
                 tc.tile_pool(name="rdp", bufs=3) as rdp, \
                 tc.tile_pool(name="psm", bufs=2) as psmp, \
                 tc.tile_pool(name="ost", bufs=10) as ostp, \
                 tc.tile_pool(name="ps_st", bufs=2, space="PSUM") as ps_st, \
                 tc.tile_pool(name="ps_acc", bufs=1, space="PSUM") as ps_acc, \
                 tc.tile_pool(name="ps3", bufs=2, space="PSUM") as ps3:
                pending = []  # deferred den/O matmuls, up to 3 k-pairs deep

                def den_o_flush(all_=False):
                    while pending and (all_ or len(pending) > 3):
                        pending.pop(0)()

                out_q = [0]

                def out_queue():
                    q = (nc.sync, nc.gpsimd, nc.scalar)[out_q[0] % 3]
                    out_q[0] += 1
                    return q

                def emit_oproj(jq, otn_sb, final=False):
                    # o-projection for q block jq (its attention is done)
                    for lt in range(4 * jq, 4 * jq + 4):
                        lo = (lt - 4 * jq) * 128
                        for et in range(D // 512):
                            es = slice(et * 512, (et + 1) * 512)
                            acc = ps3.tile([128, 512], F32, tag="acc3")
                            for h in range(HPC):
                                nc.tensor.matmul(
                                    out=acc,
                                    lhsT=otn_sb[:, h, lo:lo + 128],
                                    rhs=wo_sb[:, h, es],
                                    start=(h == 0), stop=(h == HPC - 1))
                            # copies stay off the Act engine while exps still
                            # pace the attention pipeline; the final block has
                            # no exps left, so split copies DVE/Act to halve
                            # the end-of-kernel drain
                            ost = ostp.tile([128, 512], F16, tag="ost")
                            if final and et % 2 == 1:
                                nc.scalar.copy(out=ost, in_=acc)
                            else:
                                nc.vector.tensor_copy(ost, acc)
                            out_queue().dma_start(
                                out=out_p[lt * 128:(lt + 1) * 128, es], in_=ost)

                oproj_q = []  # q blocks whose o-projection is still pending

                for jq in range(NJQ):
                    qs = slice(jq * 512, (jq + 1) * 512)
                    nkt = 4 * (jq + 1) if causal else NKT
                    npair = nkt // 2
                    diag0 = 4 * jq
                    if causal:
                        mblk = mall[:, 4 * jq:4 * jq + 4, :]
                    else:
                        mblk = mbp.tile([128, NKT, 512], F16, tag="mblk")
                        nc.sync.dma_start(
                            out=mblk,
                            in_=mask_t[:, qs].rearrange("(kt p) q -> p kt q", p=128))
                    otn_sb = otnp.tile([128, HPC, 512], F16, tag="otn")
                    for h in range(HPC):
                        qblk = qt_sb[:, h, qs]
                        den = ps_acc.tile([128, 512], F32, tag="den")
                        ot = ps_acc.tile([128, 512], F32, tag="ot")
                        for j in range(npair):
                            st = ps_st.tile([128, 2, 512], F32, tag="st")
                            pt = ptp.tile([128, 2, 512], F16, tag="pt")
                            for jj in range(2):
                                i = 2 * j + jj
                                nc.tensor.matmul(
                                    out=st[:, jj, :],
                                    lhsT=kt_sb[:, i * 128:(i + 1) * 128],
                                    rhs=qblk, start=True, stop=True)
                            # diag0 is even, so a k-pair is either fully on
                            # the diagonal block or fully below it: one
                            # contiguous [128,2,512] mask add per pair
                            if causal:
                                if 2 * j >= diag0:
                                    m0 = 2 * j - diag0
                                    nc.vector.tensor_add(
                                        st, st, mblk[:, m0:m0 + 2, :])
                            else:
                                nc.vector.tensor_add(
                                    st, st, mblk[:, 2 * j:2 * j + 2, :])
                            nc.scalar.activation(pt, st, EXP, bias=bias_col)
                            if oproj_q:
                                # previous q block's o-projection rides in the
                                # attention pipeline gap; its otn must be fully
                                # written, so drain the den/O queue first
                                den_o_flush(all_=True)
                                emit_oproj(*oproj_q.pop(0))

                            def den_o(pt=pt, j=j, den=den, ot=ot, nkt=nkt,
                                      last=(j == npair - 1), h=h, otn_sb=otn_sb):
                                for jj in range(2):
                                    i = 2 * j + jj
                                    nc.tensor.matmul(
                                        out=den, lhsT=ones, rhs=pt[:, jj, :],
                                        start=(i == 0), stop=(i == nkt - 1))
                                    nc.tensor.matmul(
                                        out=ot, lhsT=v_sb[:, i, :], rhs=pt[:, jj, :],
                                        start=(i == 0), stop=(i == nkt - 1))
                                if last:
                                    rd = rdp.tile([128, 512], F32, tag="rd")
                                    nc.vector.reciprocal_approx_fast(out=rd, in_=den)
                                    nc.vector.tensor_mul(otn_sb[:, h, :], ot, rd)

                            pending.append(den_o)
                            den_o_flush()

                    oproj_q.append((jq, otn_sb))

                den_o_flush(all_=True)
                emit_oproj(*oproj_q.pop(0), final=True)

            mbp_cm.__exit__(None, None, None)
            wop_cm.__exit__(None, None, None)

    nc.compile()
    return nc


def _host_inputs(x, attention_mask, cos, sin, w_qkv, w_o, causal):
    """Build the 8 per-core input maps (fp16, C-contiguous)."""
    F16 = np.float16
    xt = np.ascontiguousarray(x[0].T).astype(F16)         # [D, L]
    q_pos = H * HD
    kv_pos = q_pos + KV * HD

    # extended rope tables [2, 128, L]: slot 0 = q (scale folded), slot 1 = k
    # row d<64: cos[l, d]; row d>=64: 1.0 (cos) / 0.0 (sin)
    cos_t = cos.T.astype(np.float32)                      # [ROT, L]
    sin_t = sin.T.astype(np.float32)
    cos_e = np.empty((2, 128, L), np.float32)
    sin_e = np.zeros((2, 128, L), np.float32)
    cos_e[0, :ROT] = cos_t * SCALE
    cos_e[0, ROT:] = SCALE
    cos_e[1, :ROT] = cos_t
    cos_e[1, ROT:] = 1.0
    sin_e[0, :ROT] = sin_t * SCALE
    sin_e[1, :ROT] = sin_t
    cos_e = cos_e.astype(F16)
    sin_e = sin_e.astype(F16)

    # consts [128, 385] = [identity | ones | pmat_t | exp bias]
    # pmat_t[d, d'] = Pmat[d', d]; rot[d'] = -x[d'+32] (d'<32), x[d'-32] (32<=d'<64)
    pmat = np.zeros((128, 128), np.float32)
    for dp in range(32):
        pmat[dp, dp + 32] = -1.0
    for dp in range(32, 64):
        pmat[dp, dp - 32] = 1.0
    consts = np.concatenate(
        [np.eye(128, dtype=np.float32), np.ones((128, 128), np.float32), pmat.T,
         np.full((128, 1), EXP_BIAS, np.float32)],
        axis=1).astype(F16)

    # fp16-safe mask: clamp -1e9 to -30000 (exp still exactly 0 after bias)
    mask2d = np.maximum(attention_mask[0, 0], MASK_NEG)   # [L(q), L(k)]
    if causal:
        mask_t_full = None
        # diagonal 512x512 blocks of maskT, split into 128-row k strips
        mask_d = np.empty((NJQ, 4, 128, 512), F16)
        mt = mask2d.T                                     # [k, q]
        for jq in range(NJQ):
            blk = mt[jq * 512:(jq + 1) * 512, jq * 512:(jq + 1) * 512]
            mask_d[jq] = blk.reshape(4, 128, 512)
        mask_d = np.ascontiguousarray(mask_d)
    else:
        mask_t_full = np.ascontiguousarray(mask2d.T).astype(F16)  # [k, q]
        mask_d = None

    in_maps = []
    for c in range(NCORES):
        cols = []
        for j in range(HPC):
            h = c * HPC + j
            cols.append(w_qkv[:, h * HD:(h + 1) * HD])
        cols.append(w_qkv[:, q_pos + c * HD:q_pos + (c + 1) * HD])
        cols.append(w_qkv[:, kv_pos + c * HD:kv_pos + (c + 1) * HD])
        wqkv_c = np.concatenate(cols, axis=1)             # [D, 768]
        # pre-permute to [p, ct, dt, c]: contiguous per-partition chunks
        wqkv_c = np.ascontiguousarray(
            wqkv_c.reshape(NDT, 128, NCT, 128).transpose(1, 2, 0, 3)).astype(F16)
        wo_c = np.ascontiguousarray(
            w_o[c * HPC * HD:(c + 1) * HPC * HD, :]).astype(F16)     # [512, D]
        m = {"xt": xt, "wqkv": wqkv_c, "wo": wo_c,
             "cos_e": cos_e, "sin_e": sin_e, "consts": consts}
        if causal:
            m["mask_d"] = mask_d
        else:
            m["mask_t"] = mask_t_full
        in_maps.append(m)
    return in_maps


def _is_causal(mask2d):
    expected = np.where(
        np.tril(np.ones((L, L), dtype=bool)), np.float32(0.0), np.float32(NEG))
    return mask2d.shape == (L, L) and np.array_equal(mask2d, expected)


def kernel(x, attention_mask, cos, sin, w_qkv, w_o, _trace=False):
    from concourse.bass_utils import run_bass_kernel_spmd

    x = np.asarray(x, dtype=np.float32)
    attention_mask = np.asarray(attention_mask, dtype=np.float32)
    cos = np.asarray(cos, dtype=np.float32)
    sin = np.asarray(sin, dtype=np.float32)
    w_qkv = np.asarray(w_qkv, dtype=np.float32)
    w_o = np.asarray(w_o, dtype=np.float32)

    causal = _is_causal(attention_mask[0, 0])
    if causal not in _cache:
        _cache[causal] = _build(causal)
    nc = _cache[causal]

    in_maps = _host_inputs(x, attention_mask, cos, sin, w_qkv, w_o, causal)
    try:
        res = run_bass_kernel_spmd(nc, in_maps, list(range(NCORES)), trace=_trace)
    except Exception:
        # transient device errors (e.g. NRT_EXEC_UNIT_UNRECOVERABLE) usually
        # clear on retry
        res = run_bass_kernel_spmd(nc, in_maps, list(range(NCORES)), trace=_trace)
    out = np.zeros((L, D), np.float64)
    for c in range(NCORES):
        out += res.results[c]["out_p"].astype(np.float64)
    if _trace:
        kernel._last_exec_time_ns = res.exec_time_ns
    return out.astype(np.float32).reshape(B, L, D)
